# revision 2
# baseline (speedup 1.0000x reference)
"""LoRALinear fused kernel for 8 trn2 NeuronCores (v2: bf16 weight-stationary).

y = x @ (base + 2*(B@A))^T + bias,  x:[2,2048,4096], base:[4096,4096],
A:[8,4096], B:[4096,8], bias:[4096] -> y:[2,2048,4096], fp32 in/out.

Sharding: 8 token shards (512 tokens/core), weights replicated.  Per core
the output is computed transposed (d_out on partitions, tokens free):
  yT_c[4096, 512] = W^T-stationary matmuls over 32 k-chunks, where
  yT_c = base^T.T@x_c^T + [2B^T; bias].T @ [A@x_c^T ; ones].
Operands are cast to bf16 on the host (absmax rel err ~1.6e-3, fp32 PSUM
accumulation); this halves DMA vs f32r so the x/weight streams never
stall the PE.  32 o-tiles of 128 rows pipeline through 7 PSUM banks
(bank 8 holds PT=A@x^T during o-tile 0); each o-tile closes (LoRA+bias
via one K=9 matmul) two tiles after its accumulation, so evacuation
overlaps downstream compute and the PE never waits on bank reuse.
Host does layout/cast only; all FLOPs are on device.
"""
import sys

sys.path.insert(0, "/opt/trn_rl_repo")

import numpy as np
import ml_dtypes

T, D, O = 4096, 4096, 4096
N_CORES = 8
TC = T // N_CORES          # 512 tokens per core
KC = D // 128              # 32 contraction chunks
OT = O // 128              # 32 output tiles per core
WG = 8                     # k-chunks per weight DMA (4 DMAs per o-tile)

_cache = {}


def _bf16(a):
    return np.ascontiguousarray(a, dtype=np.float32).astype(ml_dtypes.bfloat16)


def _build():
    import concourse.mybir as mybir
    import concourse.tile as tile
    from concourse import bacc

    f32 = mybir.dt.float32
    bf = mybir.dt.bfloat16

    nc = bacc.Bacc("TRN2", target_bir_lowering=False, debug=False,
                   num_devices=8)

    xt_d = nc.dram_tensor("xt", [128, KC, TC], bf, kind="ExternalInput").ap()
    wt_d = nc.dram_tensor("wt", [128, OT, KC, 128], bf,
                          kind="ExternalInput").ap()
    at_d = nc.dram_tensor("at", [128, KC, 8], bf, kind="ExternalInput").ap()
    # rows 0-7: 2*B^T per o-tile, row 8: bias
    bb_d = nc.dram_tensor("bb", [9, OT, 128], bf, kind="ExternalInput").ap()
    ones_d = nc.dram_tensor("ones", [1, TC], bf, kind="ExternalInput").ap()
    y_d = nc.dram_tensor("y", [O, TC], f32, kind="ExternalOutput").ap()

    with tile.TileContext(nc) as tc:
        with (
            tc.tile_pool(name="res", bufs=1) as res,
            tc.tile_pool(name="wst", bufs=16) as wst,
            tc.tile_pool(name="evac", bufs=4) as evac,
            tc.tile_pool(name="psum", bufs=1, space="PSUM") as psum,
        ):
            # --- residents; DMA issue order per ring is the program order.
            # sync ring: at, w(ot0) split, xt odd pairs, bb, then w(ot>=1)
            # scalar ring: xt chunk0, xt even pairs, ones, then y-outs
            at = res.tile([128, KC, 8], bf)
            nc.sync.dma_start(at[:], at_d[:])
            xt = res.tile([128, KC, TC], bf)
            nc.scalar.dma_start(xt[:, 0, :], xt_d[:, 0, :])

            # o-tile 0 weights, split fine for a fast start
            w00a = wst.tile([128, 2, 128], bf, name="w00a", tag="wt0", bufs=1)
            nc.sync.dma_start(w00a[:], wt_d[:, 0, 0:2, :])
            w00b = wst.tile([128, 6, 128], bf, name="w00b", tag="wt0b", bufs=1)
            nc.sync.dma_start(w00b[:], wt_d[:, 0, 2:8, :])
            w0g = {}
            for g in range(1, 4):
                w = wst.tile([128, WG, 128], bf, name=f"w0_{g}", tag="wt")
                nc.sync.dma_start(w[:], wt_d[:, 0, g * WG:(g + 1) * WG, :])
                w0g[g] = w

            # xt chunk pairs alternate rings so both drain in parallel
            k = 1
            ring = 0
            while k < KC:
                n = min(2, KC - k)
                eng = nc.scalar if ring == 0 else nc.sync
                eng.dma_start(xt[:, k:k + n, :], xt_d[:, k:k + n, :])
                ring ^= 1
                k += n

            bb = res.tile([9, OT, 128], bf)
            nc.sync.dma_start(bb[:], bb_d[:])
            ptw = res.tile([9, TC], bf)
            nc.scalar.dma_start(ptw[8:9, :], ones_d[:])

            accs = {}

            def close_and_evac(ot, split_out=False):
                acc = accs.pop(ot)
                nc.tensor.matmul(acc[:], bb[:, ot, :], ptw[:],
                                 start=False, stop=True)
                ev = evac.tile([128, TC], f32, name=f"ev{ot % 4}", tag="ev")
                nc.vector.tensor_copy(ev[:], acc[:])
                osl = slice(128 * ot, 128 * (ot + 1))
                if split_out:
                    h = TC // 2
                    nc.scalar.dma_start(y_d[osl, 0:h], ev[:, 0:h])
                    nc.sync.dma_start(y_d[osl, h:TC], ev[:, h:TC])
                else:
                    nc.scalar.dma_start(y_d[osl, :], ev[:])

            # --- o-tile 0 fused with PT = A @ x^T (PSUM bank 8)
            acc = psum.tile([128, TC], f32, name="acc0", tag="P0")
            accs[0] = acc
            pt = psum.tile([8, TC], f32, name="pt", tag="P7")
            for k in range(KC):
                if k < 2:
                    wsl = w00a[:, k, :]
                elif k < 8:
                    wsl = w00b[:, k - 2, :]
                else:
                    wsl = w0g[k // WG][:, k % WG, :]
                nc.tensor.matmul(acc[:], wsl, xt[:, k, :],
                                 start=(k == 0), stop=False)
                nc.tensor.matmul(pt[:], at[:, k, :], xt[:, k, :],
                                 start=(k == 0), stop=(k == KC - 1))
            nc.vector.tensor_copy(ptw[0:8, :], pt[:])

            # --- o-tiles 1..31, closing ot-2 after each k-loop
            for ot in range(1, OT):
                wts = []
                for g in range(4):
                    w = wst.tile([128, WG, 128], bf, name=f"w{ot}_{g}",
                                 tag="wt")
                    nc.sync.dma_start(
                        w[:], wt_d[:, ot, g * WG:(g + 1) * WG, :])
                    wts.append(w)
                acc = psum.tile([128, TC], f32, name=f"acc{ot}",
                                tag=f"P{ot % 7}")
                accs[ot] = acc
                for g in range(4):
                    for j in range(WG):
                        k = g * WG + j
                        nc.tensor.matmul(acc[:], wts[g][:, j, :],
                                         xt[:, k, :],
                                         start=(k == 0), stop=False)
                if ot >= 2:
                    close_and_evac(ot - 2)
            close_and_evac(OT - 2)
            close_and_evac(OT - 1, split_out=True)

    nc.compile()
    return nc


def _get_nc():
    if "nc" not in _cache:
        _cache["nc"] = _build()
    return _cache["nc"]


def kernel(x, base_weight, lora_A, lora_B, bias, _trace=False, _trace_kwargs=None):
    from concourse.bass_utils import run_bass_kernel_spmd

    nc = _get_nc()

    x_flat = np.ascontiguousarray(x, dtype=np.float32).reshape(T, D)
    # wt[p, ot, kc, oo] = W[ot*128+oo, kc*128+p]
    wt = np.ascontiguousarray(
        _bf16(base_weight).reshape(OT, 128, KC, 128).transpose(3, 0, 2, 1))
    at = np.ascontiguousarray(
        _bf16(lora_A).reshape(8, KC, 128).transpose(2, 1, 0))
    bb = np.ascontiguousarray(np.concatenate([
        _bf16(2.0 * lora_B).T.reshape(8, OT, 128),
        _bf16(bias).reshape(1, OT, 128)], axis=0))
    ones = np.ones((1, TC), dtype=ml_dtypes.bfloat16)

    in_maps = []
    for c in range(N_CORES):
        xt = np.ascontiguousarray(
            _bf16(x_flat[TC * c:TC * (c + 1)]).T
            .reshape(KC, 128, TC).transpose(1, 0, 2))
        in_maps.append({"xt": xt, "wt": wt, "at": at, "bb": bb,
                        "ones": ones})

    res = run_bass_kernel_spmd(nc, in_maps, list(range(N_CORES)),
                               trace=_trace, **(_trace_kwargs or {}))

    y = np.empty((T, O), dtype=np.float32)
    for c in range(N_CORES):
        y[TC * c:TC * (c + 1), :] = res.results[c]["y"].T
    out = y.reshape(x.shape[0], x.shape[1], O)
    if _trace:
        return out, res
    return out


# revision 4
# speedup vs baseline: 1.0420x; 1.0420x over previous
"""LoRALinear fused kernel for 8 trn2 NeuronCores (v2: bf16 weight-stationary).

y = x @ (base + 2*(B@A))^T + bias,  x:[2,2048,4096], base:[4096,4096],
A:[8,4096], B:[4096,8], bias:[4096] -> y:[2,2048,4096], fp32 in/out.

Sharding: 8 token shards (512 tokens/core), weights replicated.  Per core
the output is computed transposed (d_out on partitions, tokens free):
  yT_c[4096, 512] = W^T-stationary matmuls over 32 k-chunks, where
  yT_c = base^T.T@x_c^T + [2B^T; bias].T @ [A@x_c^T ; ones].
Operands are cast to bf16 on the host (absmax rel err ~1.6e-3, fp32 PSUM
accumulation); this halves DMA vs f32r so the x/weight streams never
stall the PE.  32 o-tiles of 128 rows pipeline through 7 PSUM banks
(bank 8 holds PT=A@x^T during o-tile 0); each o-tile closes (LoRA+bias
via one K=9 matmul) two tiles after its accumulation, so evacuation
overlaps downstream compute and the PE never waits on bank reuse.
Host does layout/cast only; all FLOPs are on device.
"""
import sys

sys.path.insert(0, "/opt/trn_rl_repo")

import numpy as np
import ml_dtypes

T, D, O = 4096, 4096, 4096
N_CORES = 8
TC = T // N_CORES          # 512 tokens per core
KC = D // 128              # 32 contraction chunks
OT = O // 128              # 32 output tiles per core
WG = 8                     # k-chunks per weight DMA (4 DMAs per o-tile)

_cache = {}


def _bf16(a):
    return np.ascontiguousarray(a, dtype=np.float32).astype(ml_dtypes.bfloat16)


def _build():
    import concourse.mybir as mybir
    import concourse.tile as tile
    from concourse import bacc

    f32 = mybir.dt.float32
    bf = mybir.dt.bfloat16

    nc = bacc.Bacc("TRN2", target_bir_lowering=False, debug=False,
                   num_devices=8)

    xt_d = nc.dram_tensor("xt", [128, KC, TC], bf, kind="ExternalInput").ap()
    wt_d = nc.dram_tensor("wt", [128, OT, KC, 128], bf,
                          kind="ExternalInput").ap()
    at_d = nc.dram_tensor("at", [128, KC, 8], bf, kind="ExternalInput").ap()
    # rows 0-7: 2*B^T per o-tile, row 8: bias
    bb_d = nc.dram_tensor("bb", [9, OT, 128], bf, kind="ExternalInput").ap()
    ones_d = nc.dram_tensor("ones", [1, TC], bf, kind="ExternalInput").ap()
    y_d = nc.dram_tensor("y", [O, TC], f32, kind="ExternalOutput").ap()

    with tile.TileContext(nc) as tc:
        with (
            tc.tile_pool(name="res", bufs=1) as res,
            tc.tile_pool(name="wst", bufs=16) as wst,
            tc.tile_pool(name="evac", bufs=4) as evac,
            tc.tile_pool(name="psum", bufs=1, space="PSUM") as psum,
        ):
            # --- residents; DMA issue order per ring is the program order.
            # scalar ring: xt chunks 0..16 (early singles), ones, y-outs
            # sync ring: w(ot0) split + at, then xt 17..31 interleaved, bb,
            #            then w(ot>=1)
            xt = res.tile([128, KC, TC], bf)
            nc.scalar.dma_start(xt[:, 0, :], xt_d[:, 0, :])
            nc.scalar.dma_start(xt[:, 1, :], xt_d[:, 1, :])
            nc.scalar.dma_start(xt[:, 2, :], xt_d[:, 2, :])
            for k in range(3, 17, 2):
                nc.scalar.dma_start(xt[:, k:k + 2, :], xt_d[:, k:k + 2, :])

            # o-tile 0 weights, split fine for a fast start
            w00a = wst.tile([128, 2, 128], bf, name="w00a", tag="wt0", bufs=1)
            nc.sync.dma_start(w00a[:], wt_d[:, 0, 0:2, :])
            at = res.tile([128, KC, 8], bf)
            nc.sync.dma_start(at[:], at_d[:])
            w00b = wst.tile([128, 6, 128], bf, name="w00b", tag="wt0b", bufs=1)
            nc.sync.dma_start(w00b[:], wt_d[:, 0, 2:8, :])
            w0g = {}
            xk = 17
            for g in range(1, 4):
                w = wst.tile([128, WG, 128], bf, name=f"w0_{g}", tag="wt")
                nc.sync.dma_start(w[:], wt_d[:, 0, g * WG:(g + 1) * WG, :])
                w0g[g] = w
                nc.sync.dma_start(xt[:, xk:xk + 2, :], xt_d[:, xk:xk + 2, :])
                xk += 2
            while xk < KC:
                n = min(2, KC - xk)
                nc.sync.dma_start(xt[:, xk:xk + n, :], xt_d[:, xk:xk + n, :])
                xk += n

            bb = res.tile([9, OT, 128], bf)
            nc.sync.dma_start(bb[:], bb_d[:])
            ptw = res.tile([9, TC], bf)
            nc.scalar.dma_start(ptw[8:9, :], ones_d[:])

            accs = {}

            def close_and_evac(ot, split_out=False):
                acc = accs.pop(ot)
                nc.tensor.matmul(acc[:], bb[:, ot, :], ptw[:],
                                 start=False, stop=True)
                osl = slice(128 * ot, 128 * (ot + 1))
                if split_out:
                    h = TC // 2
                    ev = evac.tile([128, TC], f32, name=f"ev{ot % 4}",
                                   tag="ev")
                    nc.vector.tensor_copy(ev[:, 0:h], acc[:, 0:h])
                    nc.scalar.dma_start(y_d[osl, 0:h], ev[:, 0:h])
                    nc.vector.tensor_copy(ev[:, h:TC], acc[:, h:TC])
                    nc.sync.dma_start(y_d[osl, h:TC], ev[:, h:TC])
                else:
                    ev = evac.tile([128, TC], f32, name=f"ev{ot % 4}",
                                   tag="ev")
                    nc.vector.tensor_copy(ev[:], acc[:])
                    nc.scalar.dma_start(y_d[osl, :], ev[:])

            # --- o-tile 0 fused with PT = A @ x^T (PSUM bank 8); PT
            # matmuls chained in groups of 4 so they pipeline
            acc = psum.tile([128, TC], f32, name="acc0", tag="P0")
            accs[0] = acc
            pt = psum.tile([8, TC], f32, name="pt", tag="P7")
            for k4 in range(0, KC, 4):
                for k in range(k4, k4 + 4):
                    if k < 2:
                        wsl = w00a[:, k, :]
                    elif k < 8:
                        wsl = w00b[:, k - 2, :]
                    else:
                        wsl = w0g[k // WG][:, k % WG, :]
                    nc.tensor.matmul(acc[:], wsl, xt[:, k, :],
                                     start=(k == 0), stop=False)
                for k in range(k4, k4 + 4):
                    nc.tensor.matmul(pt[:], at[:, k, :], xt[:, k, :],
                                     start=(k == 0), stop=(k == KC - 1))
            nc.vector.tensor_copy(ptw[0:8, :], pt[:])

            # --- o-tiles 1..31; after each even o-tile close the two
            # oldest pending accumulators back-to-back (they pipeline)
            for ot in range(1, OT):
                wts = []
                for g in range(4):
                    w = wst.tile([128, WG, 128], bf, name=f"w{ot}_{g}",
                                 tag="wt")
                    nc.sync.dma_start(
                        w[:], wt_d[:, ot, g * WG:(g + 1) * WG, :])
                    wts.append(w)
                acc = psum.tile([128, TC], f32, name=f"acc{ot}",
                                tag=f"P{ot % 7}")
                accs[ot] = acc
                for g in range(4):
                    for j in range(WG):
                        k = g * WG + j
                        nc.tensor.matmul(acc[:], wts[g][:, j, :],
                                         xt[:, k, :],
                                         start=(k == 0), stop=False)
                if ot >= 2 and ot % 2 == 0:
                    close_and_evac(ot - 2)
                    close_and_evac(ot - 1)
            close_and_evac(OT - 2)
            close_and_evac(OT - 1, split_out=True)

    nc.compile()
    return nc


def _get_nc():
    if "nc" not in _cache:
        _cache["nc"] = _build()
    return _cache["nc"]


def kernel(x, base_weight, lora_A, lora_B, bias, _trace=False, _trace_kwargs=None):
    from concourse.bass_utils import run_bass_kernel_spmd

    nc = _get_nc()

    x_flat = np.ascontiguousarray(x, dtype=np.float32).reshape(T, D)
    # wt[p, ot, kc, oo] = W[ot*128+oo, kc*128+p]
    wt = np.ascontiguousarray(
        _bf16(base_weight).reshape(OT, 128, KC, 128).transpose(3, 0, 2, 1))
    at = np.ascontiguousarray(
        _bf16(lora_A).reshape(8, KC, 128).transpose(2, 1, 0))
    bb = np.ascontiguousarray(np.concatenate([
        _bf16(2.0 * lora_B).T.reshape(8, OT, 128),
        _bf16(bias).reshape(1, OT, 128)], axis=0))
    ones = np.ones((1, TC), dtype=ml_dtypes.bfloat16)

    in_maps = []
    for c in range(N_CORES):
        xt = np.ascontiguousarray(
            _bf16(x_flat[TC * c:TC * (c + 1)]).T
            .reshape(KC, 128, TC).transpose(1, 0, 2))
        in_maps.append({"xt": xt, "wt": wt, "at": at, "bb": bb,
                        "ones": ones})

    res = run_bass_kernel_spmd(nc, in_maps, list(range(N_CORES)),
                               trace=_trace, **(_trace_kwargs or {}))

    y = np.empty((T, O), dtype=np.float32)
    for c in range(N_CORES):
        y[TC * c:TC * (c + 1), :] = res.results[c]["y"].T
    out = y.reshape(x.shape[0], x.shape[1], O)
    if _trace:
        return out, res
    return out


# revision 6
# speedup vs baseline: 1.0502x; 1.0079x over previous
"""LoRALinear fused kernel for 8 trn2 NeuronCores (v2: bf16 weight-stationary).

y = x @ (base + 2*(B@A))^T + bias,  x:[2,2048,4096], base:[4096,4096],
A:[8,4096], B:[4096,8], bias:[4096] -> y:[2,2048,4096], fp32 in/out.

Sharding: 8 token shards (512 tokens/core), weights replicated.  Per core
the output is computed transposed (d_out on partitions, tokens free):
  yT_c[4096, 512] = W^T-stationary matmuls over 32 k-chunks, where
  yT_c = base^T.T@x_c^T + [2B^T; bias].T @ [A@x_c^T ; ones].
Operands are cast to bf16 on the host (absmax rel err ~1.6e-3, fp32 PSUM
accumulation); this halves DMA vs f32r so the x/weight streams never
stall the PE.  32 o-tiles of 128 rows pipeline through 7 PSUM banks
(bank 8 holds PT=A@x^T during o-tile 0); each o-tile closes (LoRA+bias
via one K=9 matmul) two tiles after its accumulation, so evacuation
overlaps downstream compute and the PE never waits on bank reuse.
Host does layout/cast only; all FLOPs are on device.
"""
import sys

sys.path.insert(0, "/opt/trn_rl_repo")

import numpy as np
import ml_dtypes

T, D, O = 4096, 4096, 4096
N_CORES = 8
TC = T // N_CORES          # 512 tokens per core
KC = D // 128              # 32 contraction chunks
OT = O // 128              # 32 output tiles per core
WG = 8                     # k-chunks per weight DMA (4 DMAs per o-tile)

_cache = {}


def _bf16(a):
    return np.ascontiguousarray(a, dtype=np.float32).astype(ml_dtypes.bfloat16)


def _build():
    import concourse.mybir as mybir
    import concourse.tile as tile
    from concourse import bacc

    f32 = mybir.dt.float32
    bf = mybir.dt.bfloat16

    nc = bacc.Bacc("TRN2", target_bir_lowering=False, debug=False,
                   num_devices=8)

    xt_d = nc.dram_tensor("xt", [128, KC, TC], bf, kind="ExternalInput").ap()
    wt_d = nc.dram_tensor("wt", [128, OT, KC, 128], bf,
                          kind="ExternalInput").ap()
    at_d = nc.dram_tensor("at", [128, KC, 8], bf, kind="ExternalInput").ap()
    # rows 0-7: 2*B^T per o-tile, row 8: bias
    bb_d = nc.dram_tensor("bb", [9, OT, 128], bf, kind="ExternalInput").ap()
    ones_d = nc.dram_tensor("ones", [1, TC], bf, kind="ExternalInput").ap()
    y_d = nc.dram_tensor("y", [O, TC], f32, kind="ExternalOutput").ap()

    with tile.TileContext(nc) as tc:
        with (
            tc.tile_pool(name="res", bufs=1) as res,
            tc.tile_pool(name="wst", bufs=16) as wst,
            tc.tile_pool(name="evac", bufs=4) as evac,
            tc.tile_pool(name="psum", bufs=1, space="PSUM") as psum,
        ):
            # --- residents; DMA issue order per ring is the program order.
            # scalar ring: xt chunks 0..16 (early singles), ones, y-outs
            # sync ring: w(ot0) split + at, then xt 17..31 interleaved, bb,
            #            then w(ot>=1)
            xt = res.tile([128, KC, TC], bf)
            nc.scalar.dma_start(xt[:, 0, :], xt_d[:, 0, :])
            nc.scalar.dma_start(xt[:, 1, :], xt_d[:, 1, :])
            nc.scalar.dma_start(xt[:, 2, :], xt_d[:, 2, :])
            for k in range(3, 17, 2):
                nc.scalar.dma_start(xt[:, k:k + 2, :], xt_d[:, k:k + 2, :])

            # o-tile 0 weights, split fine for a fast start
            w00a = wst.tile([128, 1, 128], bf, name="w00a", tag="wt0", bufs=1)
            nc.sync.dma_start(w00a[:], wt_d[:, 0, 0:1, :])
            at = res.tile([128, KC, 8], bf)
            nc.sync.dma_start(at[:], at_d[:])
            w00b = wst.tile([128, 7, 128], bf, name="w00b", tag="wt0b", bufs=1)
            nc.sync.dma_start(w00b[:], wt_d[:, 0, 1:8, :])
            w0g = {}
            xk = 17
            for g in range(1, 4):
                w = wst.tile([128, WG, 128], bf, name=f"w0_{g}", tag="wt")
                nc.sync.dma_start(w[:], wt_d[:, 0, g * WG:(g + 1) * WG, :])
                w0g[g] = w
                nc.sync.dma_start(xt[:, xk:xk + 2, :], xt_d[:, xk:xk + 2, :])
                xk += 2
            while xk < KC:
                n = min(2, KC - xk)
                nc.sync.dma_start(xt[:, xk:xk + n, :], xt_d[:, xk:xk + n, :])
                xk += n

            bb = res.tile([9, OT, 128], bf)
            nc.sync.dma_start(bb[:], bb_d[:])
            ptw = res.tile([9, TC], bf)
            nc.scalar.dma_start(ptw[8:9, :], ones_d[:])

            accs = {}

            def close_and_evac(ot, split_out=False):
                acc = accs.pop(ot)
                nc.tensor.matmul(acc[:], bb[:, ot, :], ptw[:],
                                 start=False, stop=True)
                osl = slice(128 * ot, 128 * (ot + 1))
                if split_out:
                    h = TC // 2
                    ev = evac.tile([128, TC], f32, name=f"ev{ot % 4}",
                                   tag="ev")
                    nc.vector.tensor_copy(ev[:, 0:h], acc[:, 0:h])
                    nc.scalar.dma_start(y_d[osl, 0:h], ev[:, 0:h])
                    nc.vector.tensor_copy(ev[:, h:TC], acc[:, h:TC])
                    nc.sync.dma_start(y_d[osl, h:TC], ev[:, h:TC])
                else:
                    ev = evac.tile([128, TC], f32, name=f"ev{ot % 4}",
                                   tag="ev")
                    nc.vector.tensor_copy(ev[:], acc[:])
                    nc.scalar.dma_start(y_d[osl, :], ev[:])

            # --- o-tile 0 fused with PT = A @ x^T (PSUM bank 8); PT
            # matmuls chained in groups of 8 so they pipeline
            acc = psum.tile([128, TC], f32, name="acc0", tag="P0")
            accs[0] = acc
            pt = psum.tile([8, TC], f32, name="pt", tag="P7")
            for k8 in range(0, KC, WG):
                for k in range(k8, k8 + WG):
                    if k < 1:
                        wsl = w00a[:, 0, :]
                    elif k < 8:
                        wsl = w00b[:, k - 1, :]
                    else:
                        wsl = w0g[k // WG][:, k % WG, :]
                    nc.tensor.matmul(acc[:], wsl, xt[:, k, :],
                                     start=(k == 0), stop=False)
                for k in range(k8, k8 + WG):
                    nc.tensor.matmul(pt[:], at[:, k, :], xt[:, k, :],
                                     start=(k == 0), stop=(k == KC - 1))
            nc.vector.tensor_copy(ptw[0:8, :], pt[:])

            # --- o-tiles 1..31; close pending accumulators in chained
            # bursts (quads mid-stream, pairs at the end) so the K=9
            # close matmuls pipeline with each other
            for ot in range(1, OT):
                wts = []
                for g in range(4):
                    w = wst.tile([128, WG, 128], bf, name=f"w{ot}_{g}",
                                 tag="wt")
                    nc.sync.dma_start(
                        w[:], wt_d[:, ot, g * WG:(g + 1) * WG, :])
                    wts.append(w)
                acc = psum.tile([128, TC], f32, name=f"acc{ot}",
                                tag=f"P{ot % 7}")
                accs[ot] = acc
                for g in range(4):
                    for j in range(WG):
                        k = g * WG + j
                        nc.tensor.matmul(acc[:], wts[g][:, j, :],
                                         xt[:, k, :],
                                         start=(k == 0), stop=False)
                if ot % 4 == 0:
                    for c in range(ot - 4, ot):
                        close_and_evac(c)
                elif ot == OT - 2:
                    close_and_evac(ot - 2)
                    close_and_evac(ot - 1)
            close_and_evac(OT - 2)
            close_and_evac(OT - 1, split_out=True)

    nc.compile()
    return nc


def _get_nc():
    if "nc" not in _cache:
        _cache["nc"] = _build()
    return _cache["nc"]


def kernel(x, base_weight, lora_A, lora_B, bias, _trace=False, _trace_kwargs=None):
    from concourse.bass_utils import run_bass_kernel_spmd

    nc = _get_nc()

    x_flat = np.ascontiguousarray(x, dtype=np.float32).reshape(T, D)
    # wt[p, ot, kc, oo] = W[ot*128+oo, kc*128+p]
    wt = np.ascontiguousarray(
        _bf16(base_weight).reshape(OT, 128, KC, 128).transpose(3, 0, 2, 1))
    at = np.ascontiguousarray(
        _bf16(lora_A).reshape(8, KC, 128).transpose(2, 1, 0))
    bb = np.ascontiguousarray(np.concatenate([
        _bf16(2.0 * lora_B).T.reshape(8, OT, 128),
        _bf16(bias).reshape(1, OT, 128)], axis=0))
    ones = np.ones((1, TC), dtype=ml_dtypes.bfloat16)

    in_maps = []
    for c in range(N_CORES):
        xt = np.ascontiguousarray(
            _bf16(x_flat[TC * c:TC * (c + 1)]).T
            .reshape(KC, 128, TC).transpose(1, 0, 2))
        in_maps.append({"xt": xt, "wt": wt, "at": at, "bb": bb,
                        "ones": ones})

    res = run_bass_kernel_spmd(nc, in_maps, list(range(N_CORES)),
                               trace=_trace, **(_trace_kwargs or {}))

    y = np.empty((T, O), dtype=np.float32)
    for c in range(N_CORES):
        y[TC * c:TC * (c + 1), :] = res.results[c]["y"].T
    out = y.reshape(x.shape[0], x.shape[1], O)
    if _trace:
        return out, res
    return out


# revision 7
# speedup vs baseline: 1.0528x; 1.0025x over previous
"""LoRALinear fused kernel for 8 trn2 NeuronCores (v2: bf16 weight-stationary).

y = x @ (base + 2*(B@A))^T + bias,  x:[2,2048,4096], base:[4096,4096],
A:[8,4096], B:[4096,8], bias:[4096] -> y:[2,2048,4096], fp32 in/out.

Sharding: 8 token shards (512 tokens/core), weights replicated.  Per core
the output is computed transposed (d_out on partitions, tokens free):
  yT_c[4096, 512] = W^T-stationary matmuls over 32 k-chunks, where
  yT_c = base^T.T@x_c^T + [2B^T; bias].T @ [A@x_c^T ; ones].
Operands are cast to bf16 on the host (absmax rel err ~1.6e-3, fp32 PSUM
accumulation); this halves DMA vs f32r so the x/weight streams never
stall the PE.  32 o-tiles of 128 rows pipeline through 7 PSUM banks
(bank 8 holds PT=A@x^T during o-tile 0); each o-tile closes (LoRA+bias
via one K=9 matmul) two tiles after its accumulation, so evacuation
overlaps downstream compute and the PE never waits on bank reuse.
Host does layout/cast only; all FLOPs are on device.
"""
import sys

sys.path.insert(0, "/opt/trn_rl_repo")

import numpy as np
import ml_dtypes

T, D, O = 4096, 4096, 4096
N_CORES = 8
TC = T // N_CORES          # 512 tokens per core
KC = D // 128              # 32 contraction chunks
OT = O // 128              # 32 output tiles per core
WG = 8                     # k-chunks per weight DMA (4 DMAs per o-tile)

_cache = {}


def _bf16(a):
    return np.ascontiguousarray(a, dtype=np.float32).astype(ml_dtypes.bfloat16)


def _build():
    import concourse.mybir as mybir
    import concourse.tile as tile
    from concourse import bacc

    f32 = mybir.dt.float32
    bf = mybir.dt.bfloat16

    nc = bacc.Bacc("TRN2", target_bir_lowering=False, debug=False,
                   num_devices=8)

    xt_d = nc.dram_tensor("xt", [128, KC, TC], bf, kind="ExternalInput").ap()
    wt_d = nc.dram_tensor("wt", [128, OT, KC, 128], bf,
                          kind="ExternalInput").ap()
    at_d = nc.dram_tensor("at", [128, KC, 8], bf, kind="ExternalInput").ap()
    # rows 0-7: 2*B^T per o-tile, row 8: bias
    bb_d = nc.dram_tensor("bb", [9, OT, 128], bf, kind="ExternalInput").ap()
    ones_d = nc.dram_tensor("ones", [1, TC], bf, kind="ExternalInput").ap()
    y_d = nc.dram_tensor("y", [O, TC], f32, kind="ExternalOutput").ap()

    with tile.TileContext(nc) as tc:
        with (
            tc.tile_pool(name="res", bufs=1) as res,
            tc.tile_pool(name="wst", bufs=16) as wst,
            tc.tile_pool(name="evac", bufs=4) as evac,
            tc.tile_pool(name="psum", bufs=1, space="PSUM") as psum,
        ):
            # --- residents; DMA issue order per ring is the program order.
            # scalar ring: xt chunks 0..16 (early singles), ones, y-outs
            # sync ring: w(ot0) split + at, then xt 17..31 interleaved, bb,
            #            then w(ot>=1)
            xt = res.tile([128, KC, TC], bf)
            nc.scalar.dma_start(xt[:, 0, :], xt_d[:, 0, :])
            nc.scalar.dma_start(xt[:, 1, :], xt_d[:, 1, :])
            nc.scalar.dma_start(xt[:, 2, :], xt_d[:, 2, :])
            for k in range(3, 17, 2):
                nc.scalar.dma_start(xt[:, k:k + 2, :], xt_d[:, k:k + 2, :])

            # o-tile 0 weights, split fine for a fast start
            w00a = wst.tile([128, 1, 128], bf, name="w00a", tag="wt0", bufs=1)
            nc.sync.dma_start(w00a[:], wt_d[:, 0, 0:1, :])
            at = res.tile([128, KC, 8], bf)
            nc.sync.dma_start(at[:], at_d[:])
            w00b = wst.tile([128, 7, 128], bf, name="w00b", tag="wt0b", bufs=1)
            nc.sync.dma_start(w00b[:], wt_d[:, 0, 1:8, :])
            w0g = {}
            xk = 17
            for g in range(1, 4):
                w = wst.tile([128, WG, 128], bf, name=f"w0_{g}", tag="wt")
                nc.sync.dma_start(w[:], wt_d[:, 0, g * WG:(g + 1) * WG, :])
                w0g[g] = w
                nc.sync.dma_start(xt[:, xk:xk + 2, :], xt_d[:, xk:xk + 2, :])
                xk += 2
            while xk < KC:
                n = min(2, KC - xk)
                nc.sync.dma_start(xt[:, xk:xk + n, :], xt_d[:, xk:xk + n, :])
                xk += n

            bb = res.tile([9, OT, 128], bf)
            nc.sync.dma_start(bb[:], bb_d[:])
            ptw = res.tile([9, TC], bf)
            nc.scalar.dma_start(ptw[8:9, :], ones_d[:])

            # HAM warmup: dummy matmuls on a memset tile keep the PE busy
            # through the ~3.4us un-throttle window while the first real
            # operands are still in flight, so real matmuls start at 2.4GHz
            warm = res.tile([128, TC], bf)
            nc.gpsimd.memset(warm[:], 1.0)
            wacc = psum.tile([64, TC], f32, name="warm_acc", tag="P7")
            for _ in range(8):
                nc.tensor.matmul(wacc[:], warm[:, 0:64], warm[:],
                                 start=True, stop=True)

            accs = {}

            def close_and_evac(ot, split_out=False):
                acc = accs.pop(ot)
                nc.tensor.matmul(acc[:], bb[:, ot, :], ptw[:],
                                 start=False, stop=True)
                osl = slice(128 * ot, 128 * (ot + 1))
                if split_out:
                    h = TC // 2
                    ev = evac.tile([128, TC], f32, name=f"ev{ot % 4}",
                                   tag="ev")
                    nc.vector.tensor_copy(ev[:, 0:h], acc[:, 0:h])
                    nc.scalar.dma_start(y_d[osl, 0:h], ev[:, 0:h])
                    nc.vector.tensor_copy(ev[:, h:TC], acc[:, h:TC])
                    nc.sync.dma_start(y_d[osl, h:TC], ev[:, h:TC])
                else:
                    ev = evac.tile([128, TC], f32, name=f"ev{ot % 4}",
                                   tag="ev")
                    nc.vector.tensor_copy(ev[:], acc[:])
                    nc.scalar.dma_start(y_d[osl, :], ev[:])

            # --- o-tile 0 fused with PT = A @ x^T (PSUM bank 8); PT
            # matmuls chained in groups of 8 so they pipeline
            acc = psum.tile([128, TC], f32, name="acc0", tag="P0")
            accs[0] = acc
            pt = psum.tile([8, TC], f32, name="pt", tag="P7")
            for k8 in range(0, KC, WG):
                for k in range(k8, k8 + WG):
                    if k < 1:
                        wsl = w00a[:, 0, :]
                    elif k < 8:
                        wsl = w00b[:, k - 1, :]
                    else:
                        wsl = w0g[k // WG][:, k % WG, :]
                    nc.tensor.matmul(acc[:], wsl, xt[:, k, :],
                                     start=(k == 0), stop=False)
                for k in range(k8, k8 + WG):
                    nc.tensor.matmul(pt[:], at[:, k, :], xt[:, k, :],
                                     start=(k == 0), stop=(k == KC - 1))
            nc.vector.tensor_copy(ptw[0:8, :], pt[:])

            # --- o-tiles 1..31; close pending accumulators in chained
            # bursts (quads mid-stream, pairs at the end) so the K=9
            # close matmuls pipeline with each other
            for ot in range(1, OT):
                wts = []
                for g in range(4):
                    w = wst.tile([128, WG, 128], bf, name=f"w{ot}_{g}",
                                 tag="wt")
                    nc.sync.dma_start(
                        w[:], wt_d[:, ot, g * WG:(g + 1) * WG, :])
                    wts.append(w)
                acc = psum.tile([128, TC], f32, name=f"acc{ot}",
                                tag=f"P{ot % 7}")
                accs[ot] = acc
                for g in range(4):
                    for j in range(WG):
                        k = g * WG + j
                        nc.tensor.matmul(acc[:], wts[g][:, j, :],
                                         xt[:, k, :],
                                         start=(k == 0), stop=False)
                if ot % 4 == 0:
                    for c in range(ot - 4, ot):
                        close_and_evac(c)
                elif ot == OT - 2:
                    close_and_evac(ot - 2)
                    close_and_evac(ot - 1)
            close_and_evac(OT - 2)
            close_and_evac(OT - 1, split_out=True)

    nc.compile()
    return nc


def _get_nc():
    if "nc" not in _cache:
        _cache["nc"] = _build()
    return _cache["nc"]


def kernel(x, base_weight, lora_A, lora_B, bias, _trace=False, _trace_kwargs=None):
    from concourse.bass_utils import run_bass_kernel_spmd

    nc = _get_nc()

    x_flat = np.ascontiguousarray(x, dtype=np.float32).reshape(T, D)
    # wt[p, ot, kc, oo] = W[ot*128+oo, kc*128+p]
    wt = np.ascontiguousarray(
        _bf16(base_weight).reshape(OT, 128, KC, 128).transpose(3, 0, 2, 1))
    at = np.ascontiguousarray(
        _bf16(lora_A).reshape(8, KC, 128).transpose(2, 1, 0))
    bb = np.ascontiguousarray(np.concatenate([
        _bf16(2.0 * lora_B).T.reshape(8, OT, 128),
        _bf16(bias).reshape(1, OT, 128)], axis=0))
    ones = np.ones((1, TC), dtype=ml_dtypes.bfloat16)

    in_maps = []
    for c in range(N_CORES):
        xt = np.ascontiguousarray(
            _bf16(x_flat[TC * c:TC * (c + 1)]).T
            .reshape(KC, 128, TC).transpose(1, 0, 2))
        in_maps.append({"xt": xt, "wt": wt, "at": at, "bb": bb,
                        "ones": ones})

    res = run_bass_kernel_spmd(nc, in_maps, list(range(N_CORES)),
                               trace=_trace, **(_trace_kwargs or {}))

    y = np.empty((T, O), dtype=np.float32)
    for c in range(N_CORES):
        y[TC * c:TC * (c + 1), :] = res.results[c]["y"].T
    out = y.reshape(x.shape[0], x.shape[1], O)
    if _trace:
        return out, res
    return out
